# revision 11
# baseline (speedup 1.0000x reference)
"""Multi-head attention block (B=2, N=2048, C=1024, H=16, D=64) on 8 TRN2 cores.

Sharding: core c -> batch b = c // 4, head-group hg = c % 4 (4 heads per core).
Each core:
  qkvT = W_hg @ x_b^T           (fp32r matmuls, xT prepared on host)
  S^T  = kT^T q-chunks          (K=64, two heads row-packed per PE pass)
  P^T  = exp(S^T)               (ScalarE, no max-subtraction: scores ~ N(0,1))
  O^T|Z = [V|1]^T @ P^T         (accumulated over key tiles in PSUM)
  O^T /= Z                      (reciprocal + K=1 broadcast matmul + DVE mul)
  y_partial = O^T^T @ Wp^T      (pair-stacked K=128 matmuls, written to DRAM)
Host sums the 4 head-group partials per batch and adds bias.
"""

import numpy as np

import concourse.bass as bass
import concourse.tile as tile
from concourse import bacc, mybir

F32 = mybir.dt.float32
F32R = mybir.dt.float32r
EXP = mybir.ActivationFunctionType.Exp

B, S, C = 2, 2048, 1024
H, D = 16, 64
HPC = 4            # heads per core
NCT = C // 128     # 8 contraction tiles
MT = S // 128      # 16 key/seq tiles
NCH = S // 512     # 4 query chunks
# fp32r is simulated as exact fp32 in CoreSim; on HW it is the fast 4-byte
# matmul path (1 cyc/row at N>=256). Set to F32 as a (4x slower) fallback.
MM_DT = F32R


def build_bass():
    nc = bacc.Bacc("TRN2", target_bir_lowering=False)

    xt_d = nc.dram_tensor("xt", [C, S], F32R, kind="ExternalInput")
    wqk_d = nc.dram_tensor("wqk", [C, 512], F32R, kind="ExternalInput")
    wv_d = nc.dram_tensor("wv", [C, 256], F32R, kind="ExternalInput")
    wp_d = nc.dram_tensor("wp", [256, C], F32R, kind="ExternalInput")
    y_d = nc.dram_tensor("y", [S, C], F32, kind="ExternalOutput")

    def mm(out, lhsT, rhs, start, stop):
        nc.tensor.matmul(out, lhsT, rhs, start=start, stop=stop)

    with tile.TileContext(nc) as tc:
        with tc.tile_pool(name="persist", bufs=1) as persist:
            # qkT f-tiles: 0=q_h0|q_h1, 1=q_h2|q_h3, 2=k_h0|k_h1, 3=k_h2|k_h3
            qk_sb = persist.tile([128, 4 * S], MM_DT, tag="qk")
            # V augmented per key tile: [v_h0|1|v_h1|1|v_h2|1|v_h3|1] = 260 cols
            vaug = persist.tile([128, MT * 260], MM_DT, tag="vaug")
            wp_sb = persist.tile([64, HPC * C], MM_DT, tag="wp")
            ones_sb = persist.tile([128, 64], F32, tag="ones")
            onorm = [
                persist.tile([64, S], MM_DT, tag=f"onorm{h}", name=f"onorm{h}")
                for h in range(HPC)
            ]

            nc.vector.memset(ones_sb, 1.0)
            for h in range(HPC):
                nc.sync.dma_start(
                    out=wp_sb[:, h * C : (h + 1) * C],
                    in_=wp_d[h * 64 : (h + 1) * 64, :],
                )

            # ---------------- phase A: QKV projections ----------------
            with (
                tc.tile_pool(name="ph_a", bufs=1) as ph_a,
                tc.tile_pool(name="ps_qk", bufs=4, space="PSUM") as ps_qk,
                tc.tile_pool(name="ps_v", bufs=4, space="PSUM") as ps_v,
            ):
                xt_sb = ph_a.tile([128, NCT * S], MM_DT, tag="xt")
                vones = ph_a.tile([128, 260], F32, tag="vones")
                nc.vector.memset(vones, 1.0)
                for st in range(MT):
                    nc.vector.tensor_copy(
                        vaug[:, st * 260 : (st + 1) * 260], vones
                    )
                wqk_sb = ph_a.tile([128, NCT * 512], MM_DT, tag="wqk")
                wv_sb = ph_a.tile([128, NCT * 256], MM_DT, tag="wv")
                for ct in range(NCT):
                    nc.sync.dma_start(
                        out=xt_sb[:, ct * S : (ct + 1) * S],
                        in_=xt_d[ct * 128 : (ct + 1) * 128, :],
                    )
                    nc.sync.dma_start(
                        out=wqk_sb[:, ct * 512 : (ct + 1) * 512],
                        in_=wqk_d[ct * 128 : (ct + 1) * 128, :],
                    )
                    nc.sync.dma_start(
                        out=wv_sb[:, ct * 256 : (ct + 1) * 256],
                        in_=wv_d[ct * 128 : (ct + 1) * 128, :],
                    )

                # qkT[f] = wqk[:, f-block]^T @ xT   -> [128, S] per f-tile
                for f in range(4):
                    qps = [
                        ps_qk.tile([128, 512], F32, tag="qkps", name=f"qps{f}_{i}")
                        for i in range(4)
                    ]
                    for ct in range(NCT):
                        for sc_ in range(4):
                            mm(
                                qps[sc_],
                                wqk_sb[:, ct * 512 + f * 128 : ct * 512 + (f + 1) * 128],
                                xt_sb[:, ct * S + sc_ * 512 : ct * S + (sc_ + 1) * 512],
                                start=(ct == 0),
                                stop=(ct == NCT - 1),
                            )
                    for sc_ in range(4):
                        nc.vector.tensor_copy(
                            qk_sb[:, f * S + sc_ * 512 : f * S + (sc_ + 1) * 512],
                            qps[sc_],
                        )

                # V[st] = xT[:, st-block]^T @ wv   -> [128, 256] natural layout
                for st in range(MT):
                    vps = ps_v.tile([128, 256], F32, tag="vps")
                    for ct in range(NCT):
                        mm(
                            vps,
                            xt_sb[:, ct * S + st * 128 : ct * S + (st + 1) * 128],
                            wv_sb[:, ct * 256 : (ct + 1) * 256],
                            start=(ct == 0),
                            stop=(ct == NCT - 1),
                        )
                    for h in range(HPC):
                        nc.vector.tensor_copy(
                            vaug[:, st * 260 + 65 * h : st * 260 + 65 * h + 64],
                            vps[:, 64 * h : 64 * h + 64],
                        )

            # ---------------- phase B/C: attention ----------------
            with (
                tc.tile_pool(name="pt", bufs=2) as pt_pool,
                tc.tile_pool(name="small", bufs=2) as small,
                tc.tile_pool(name="ps_s", bufs=2, space="PSUM") as ps_s,
                tc.tile_pool(name="ps_o", bufs=2, space="PSUM") as ps_o,
            ):
                groups = [(0, 3), (3, 3), (6, 3), (9, 3), (12, 3), (15, 1)]
                for pr in range(2):
                    qf, kf = pr, 2 + pr
                    hA, hB = 2 * pr, 2 * pr + 1
                    for ch in range(NCH):
                        ptA = pt_pool.tile([128, MT * 512], MM_DT, tag="pt")
                        ptB = pt_pool.tile([128, MT * 512], MM_DT, tag="pt")
                        oA = ps_o.tile([128, 512], F32, tag="ops")
                        oB = ps_o.tile([128, 512], F32, tag="ops")
                        for g0, gn in groups:
                            sA = ps_s.tile([128, 1536], F32, tag="sps")
                            sB = ps_s.tile([128, 1536], F32, tag="sps")
                            for j in range(gn):
                                m = g0 + j
                                # two heads row-packed: A in PE rows 0-63,
                                # B in rows 64-127 (base_partition-derived)
                                mm(
                                    sA[:, j * 512 : (j + 1) * 512],
                                    qk_sb[0:64, kf * S + m * 128 : kf * S + (m + 1) * 128],
                                    qk_sb[0:64, qf * S + ch * 512 : qf * S + (ch + 1) * 512],
                                    start=True,
                                    stop=True,
                                )
                                mm(
                                    sB[:, j * 512 : (j + 1) * 512],
                                    qk_sb[64:128, kf * S + m * 128 : kf * S + (m + 1) * 128],
                                    qk_sb[64:128, qf * S + ch * 512 : qf * S + (ch + 1) * 512],
                                    start=True,
                                    stop=True,
                                )
                            nc.scalar.activation(
                                ptA[:, g0 * 512 : (g0 + gn) * 512],
                                sA[:, 0 : gn * 512],
                                EXP,
                            )
                            nc.scalar.activation(
                                ptB[:, g0 * 512 : (g0 + gn) * 512],
                                sB[:, 0 : gn * 512],
                                EXP,
                            )
                            for j in range(gn):
                                m = g0 + j
                                mm(
                                    oA[0:65, :],
                                    vaug[:, m * 260 + 65 * hA : m * 260 + 65 * hA + 65],
                                    ptA[:, m * 512 : (m + 1) * 512],
                                    start=(m == 0),
                                    stop=(m == MT - 1),
                                )
                                mm(
                                    oB[0:65, :],
                                    vaug[:, m * 260 + 65 * hB : m * 260 + 65 * hB + 65],
                                    ptB[:, m * 512 : (m + 1) * 512],
                                    start=(m == 0),
                                    stop=(m == MT - 1),
                                )
                        # normalize: row 64 of o[AB] is Z = sum_m exp(S^T)
                        for hx, ops in ((hA, oA), (hB, oB)):
                            rz = small.tile([128, 512], F32, tag="rz", name=f"rz{hx}")
                            nc.vector.reciprocal(
                                out=rz[64:65, :], in_=ops[64:65, :]
                            )
                            bc = ps_y.tile([128, 512], F32, tag="yps", name=f"bc{hx}")
                            nc.tensor.matmul(
                                bc[0:64, :],
                                ones_sb[64:65, 0:64],
                                rz[64:65, :],
                                start=True,
                                stop=True,
                            )
                            bcs = small.tile([128, 512], F32, tag="bcs", name=f"bcs{hx}")
                            nc.vector.tensor_copy(bcs[0:64, :], bc[0:64, :])
                            nc.vector.tensor_mul(
                                onorm[hx][:, ch * 512 : (ch + 1) * 512],
                                ops[0:64, :],
                                bcs[0:64, :],
                            )

            # ---------------- phase D: output projection ----------------
            with (
                tc.tile_pool(name="ps_y", bufs=6, space="PSUM") as ps_y,
                tc.tile_pool(name="yout", bufs=4) as yout,
            ):
                for st in range(MT):
                    for fc in range(2):
                        yps = ps_y.tile([128, 512], F32, tag="yps")
                        for h in range(HPC):
                            mm(
                                yps,
                                onorm[h][:, st * 128 : (st + 1) * 128],
                                wp_sb[:, h * C + fc * 512 : h * C + (fc + 1) * 512],
                                start=(h == 0),
                                stop=(h == HPC - 1),
                            )
                        ysb = yout.tile([128, 512], F32, tag="ysb")
                        nc.vector.tensor_copy(ysb, yps)
                        nc.sync.dma_start(
                            out=y_d[st * 128 : (st + 1) * 128, fc * 512 : (fc + 1) * 512],
                            in_=ysb,
                        )

    nc.compile()
    return nc


def make_core_inputs(x, Wqkv, Wproj):
    """Per-core input dicts. Core c: batch c//4, heads 4*(c%4) .. 4*(c%4)+3."""
    scale = D**-0.5
    xts = [np.ascontiguousarray(x[b].T).astype(np.float32) for b in range(B)]
    in_maps = []
    for core in range(8):
        b, hg = core // 4, core % 4
        heads = [HPC * hg + i for i in range(HPC)]
        rows_q = np.concatenate([Wqkv[D * h : D * (h + 1)] for h in heads]) * scale
        rows_k = np.concatenate([Wqkv[C + D * h : C + D * (h + 1)] for h in heads])
        wqk = np.ascontiguousarray(np.concatenate([rows_q, rows_k]).T, dtype=np.float32)
        wv = np.ascontiguousarray(
            np.concatenate([Wqkv[2 * C + D * h : 2 * C + D * (h + 1)] for h in heads]).T,
            dtype=np.float32,
        )
        wp = np.ascontiguousarray(
            np.concatenate([Wproj[:, D * h : D * (h + 1)] for h in heads], axis=1).T,
            dtype=np.float32,
        )
        in_maps.append({"xt": xts[b], "wqk": wqk, "wv": wv, "wp": wp})
    return in_maps


_EXEC_CACHE = {}


def _get_executor():
    """Build + jit the 8-core SPMD executable once per process."""
    if "fn" in _EXEC_CACHE:
        return _EXEC_CACHE
    import jax
    from jax.sharding import Mesh, PartitionSpec
    from jax.experimental.shard_map import shard_map
    from concourse import bass2jax
    from concourse.bass2jax import _bass_exec_p, partition_id_tensor

    nc = build_bass()
    bass2jax.install_neuronx_cc_hook()
    pid = nc.partition_id_tensor.name if nc.partition_id_tensor else None
    in_names, out_names, out_avals = [], [], []
    for alloc in nc.m.functions[0].allocations:
        if not isinstance(alloc, mybir.MemoryLocationSet):
            continue
        name = alloc.memorylocations[0].name
        if alloc.kind == "ExternalInput":
            if name != pid:
                in_names.append(name)
        elif alloc.kind == "ExternalOutput":
            out_names.append(name)
            out_avals.append(
                jax.core.ShapedArray(
                    tuple(alloc.tensor_shape), mybir.dt.np(alloc.dtype)
                )
            )
    n_params = len(in_names)
    all_names = list(in_names) + list(out_names) + ([pid] if pid else [])

    def body(*args):
        *ins, yb = args
        operands = list(ins) + [yb]
        if pid:
            operands.append(partition_id_tensor())
        outs = _bass_exec_p.bind(
            *operands,
            out_avals=tuple(out_avals),
            in_names=tuple(all_names),
            out_names=tuple(out_names),
            lowering_input_output_aliases=(),
            sim_require_finite=True,
            sim_require_nnan=True,
            nc=nc,
        )
        return outs[0]

    mesh = Mesh(np.asarray(jax.devices()[:8]), ("core",))
    fn = jax.jit(
        shard_map(
            body,
            mesh=mesh,
            in_specs=(PartitionSpec("core"),) * (n_params + 1),
            out_specs=PartitionSpec("core"),
            check_rep=False,
        ),
        donate_argnums=(n_params,),
    )
    _EXEC_CACHE.update(fn=fn, in_names=in_names)
    return _EXEC_CACHE


def kernel(x, Wqkv, Wproj, bproj):
    x = np.asarray(x, dtype=np.float32)
    Wqkv = np.asarray(Wqkv, dtype=np.float32)
    Wproj = np.asarray(Wproj, dtype=np.float32)
    bproj = np.asarray(bproj, dtype=np.float32)

    ex = _get_executor()
    in_maps = make_core_inputs(x, Wqkv, Wproj)
    glob_ins = [
        np.concatenate([np.asarray(m[name]) for m in in_maps], axis=0)
        for name in ex["in_names"]
    ]
    y0 = np.zeros((8 * S, C), np.float32)
    out = np.asarray(ex["fn"](*glob_ins, y0))  # [8*S, C]

    y = np.zeros((B, S, C), dtype=np.float32)
    for core in range(8):
        y[core // 4] += out[core * S : (core + 1) * S, :]
    y += bproj
    return y


# revision 12
# speedup vs baseline: 1.0085x; 1.0085x over previous
"""Multi-head attention block (B=2, N=2048, C=1024, H=16, D=64) on 8 TRN2 cores.

Sharding: core c -> batch b = c // 4, head-group hg = c % 4 (4 heads per core).
Each core:
  qkvT = W_hg @ x_b^T           (fp32r matmuls, xT prepared on host)
  S^T  = kT^T q-chunks          (K=64, two heads row-packed per PE pass)
  P^T  = exp(S^T)               (ScalarE, no max-subtraction: scores ~ N(0,1))
  O^T|Z = [V|1]^T @ P^T         (accumulated over key tiles in PSUM)
  O^T /= Z                      (reciprocal + K=1 broadcast matmul + DVE mul)
  y_partial = O^T^T @ Wp^T      (pair-stacked K=128 matmuls, written to DRAM)
Host sums the 4 head-group partials per batch and adds bias.
"""

import numpy as np

import concourse.bass as bass
import concourse.tile as tile
from concourse import bacc, mybir

F32 = mybir.dt.float32
F32R = mybir.dt.float32r
EXP = mybir.ActivationFunctionType.Exp

B, S, C = 2, 2048, 1024
H, D = 16, 64
HPC = 4            # heads per core
NCT = C // 128     # 8 contraction tiles
MT = S // 128      # 16 key/seq tiles
NCH = S // 512     # 4 query chunks
# fp32r is simulated as exact fp32 in CoreSim; on HW it is the fast 4-byte
# matmul path (1 cyc/row at N>=256). Set to F32 as a (4x slower) fallback.
MM_DT = F32R


def build_bass():
    nc = bacc.Bacc("TRN2", target_bir_lowering=False)

    xt_d = nc.dram_tensor("xt", [C, S], F32R, kind="ExternalInput")
    wqk_d = nc.dram_tensor("wqk", [C, 512], F32R, kind="ExternalInput")
    wv_d = nc.dram_tensor("wv", [C, 256], F32R, kind="ExternalInput")
    wp_d = nc.dram_tensor("wp", [256, C], F32R, kind="ExternalInput")
    y_d = nc.dram_tensor("y", [S, C], F32, kind="ExternalOutput")

    def mm(out, lhsT, rhs, start, stop):
        nc.tensor.matmul(out, lhsT, rhs, start=start, stop=stop)

    with tile.TileContext(nc) as tc:
        with tc.tile_pool(name="persist", bufs=1) as persist:
            # qkT f-tiles: 0=q_h0|q_h1, 1=q_h2|q_h3, 2=k_h0|k_h1, 3=k_h2|k_h3
            qk_sb = persist.tile([128, 4 * S], MM_DT, tag="qk")
            # V augmented per key tile: [v_h0|1|v_h1|1|v_h2|1|v_h3|1] = 260 cols
            vaug = persist.tile([128, MT * 260], MM_DT, tag="vaug")
            wp_sb = persist.tile([64, HPC * C], MM_DT, tag="wp")
            ones_sb = persist.tile([128, 64], F32, tag="ones")
            onorm = [
                persist.tile([64, S], MM_DT, tag=f"onorm{h}", name=f"onorm{h}")
                for h in range(HPC)
            ]

            nc.vector.memset(ones_sb, 1.0)
            for h in range(HPC):
                nc.sync.dma_start(
                    out=wp_sb[:, h * C : (h + 1) * C],
                    in_=wp_d[h * 64 : (h + 1) * 64, :],
                )

            # ---------------- phase A: QKV projections ----------------
            with (
                tc.tile_pool(name="ph_a", bufs=1) as ph_a,
                tc.tile_pool(name="ps_qk", bufs=4, space="PSUM") as ps_qk,
                tc.tile_pool(name="ps_v", bufs=4, space="PSUM") as ps_v,
            ):
                xt_sb = ph_a.tile([128, NCT * S], MM_DT, tag="xt")
                vones = ph_a.tile([128, 260], F32, tag="vones")
                nc.vector.memset(vones, 1.0)
                for st in range(MT):
                    nc.vector.tensor_copy(
                        vaug[:, st * 260 : (st + 1) * 260], vones
                    )
                wqk_sb = ph_a.tile([128, NCT * 512], MM_DT, tag="wqk")
                wv_sb = ph_a.tile([128, NCT * 256], MM_DT, tag="wv")
                for ct in range(NCT):
                    nc.sync.dma_start(
                        out=xt_sb[:, ct * S : (ct + 1) * S],
                        in_=xt_d[ct * 128 : (ct + 1) * 128, :],
                    )
                    nc.sync.dma_start(
                        out=wqk_sb[:, ct * 512 : (ct + 1) * 512],
                        in_=wqk_d[ct * 128 : (ct + 1) * 128, :],
                    )
                    nc.sync.dma_start(
                        out=wv_sb[:, ct * 256 : (ct + 1) * 256],
                        in_=wv_d[ct * 128 : (ct + 1) * 128, :],
                    )

                # qkT[f] = wqk[:, f-block]^T @ xT   -> [128, S] per f-tile
                for f in range(4):
                    qps = [
                        ps_qk.tile([128, 512], F32, tag="qkps", name=f"qps{f}_{i}")
                        for i in range(4)
                    ]
                    for ct in range(NCT):
                        for sc_ in range(4):
                            mm(
                                qps[sc_],
                                wqk_sb[:, ct * 512 + f * 128 : ct * 512 + (f + 1) * 128],
                                xt_sb[:, ct * S + sc_ * 512 : ct * S + (sc_ + 1) * 512],
                                start=(ct == 0),
                                stop=(ct == NCT - 1),
                            )
                    for sc_ in range(4):
                        nc.vector.tensor_copy(
                            qk_sb[:, f * S + sc_ * 512 : f * S + (sc_ + 1) * 512],
                            qps[sc_],
                        )

                # V[st] = xT[:, st-block]^T @ wv   -> [128, 256] natural layout
                for st in range(MT):
                    vps = ps_v.tile([128, 256], F32, tag="vps")
                    for ct in range(NCT):
                        mm(
                            vps,
                            xt_sb[:, ct * S + st * 128 : ct * S + (st + 1) * 128],
                            wv_sb[:, ct * 256 : (ct + 1) * 256],
                            start=(ct == 0),
                            stop=(ct == NCT - 1),
                        )
                    for h in range(HPC):
                        nc.vector.tensor_copy(
                            vaug[:, st * 260 + 65 * h : st * 260 + 65 * h + 64],
                            vps[:, 64 * h : 64 * h + 64],
                        )

            # ---------------- phase B/C: attention ----------------
            with (
                tc.tile_pool(name="pt", bufs=2) as pt_pool,
                tc.tile_pool(name="small", bufs=4) as small,
                tc.tile_pool(name="ps_s", bufs=2, space="PSUM") as ps_s,
                tc.tile_pool(name="ps_o", bufs=2, space="PSUM") as ps_o,
            ):
                groups = [(0, 3), (3, 3), (6, 3), (9, 3), (12, 3), (15, 1)]
                for pr in range(2):
                    qf, kf = pr, 2 + pr
                    hA, hB = 2 * pr, 2 * pr + 1
                    for ch in range(NCH):
                        ptA = pt_pool.tile([128, MT * 512], MM_DT, tag="pt")
                        ptB = pt_pool.tile([128, MT * 512], MM_DT, tag="pt")
                        oA = ps_o.tile([128, 512], F32, tag="ops")
                        oB = ps_o.tile([128, 512], F32, tag="ops")
                        for g0, gn in groups:
                            sA = ps_s.tile([128, 1536], F32, tag="sps")
                            sB = ps_s.tile([128, 1536], F32, tag="sps")
                            for j in range(gn):
                                m = g0 + j
                                # two heads row-packed: A in PE rows 0-63,
                                # B in rows 64-127 (base_partition-derived)
                                mm(
                                    sA[:, j * 512 : (j + 1) * 512],
                                    qk_sb[0:64, kf * S + m * 128 : kf * S + (m + 1) * 128],
                                    qk_sb[0:64, qf * S + ch * 512 : qf * S + (ch + 1) * 512],
                                    start=True,
                                    stop=True,
                                )
                                mm(
                                    sB[:, j * 512 : (j + 1) * 512],
                                    qk_sb[64:128, kf * S + m * 128 : kf * S + (m + 1) * 128],
                                    qk_sb[64:128, qf * S + ch * 512 : qf * S + (ch + 1) * 512],
                                    start=True,
                                    stop=True,
                                )
                            nc.scalar.activation(
                                ptA[:, g0 * 512 : (g0 + gn) * 512],
                                sA[:, 0 : gn * 512],
                                EXP,
                            )
                            nc.scalar.activation(
                                ptB[:, g0 * 512 : (g0 + gn) * 512],
                                sB[:, 0 : gn * 512],
                                EXP,
                            )
                            for j in range(gn):
                                m = g0 + j
                                mm(
                                    oA[0:65, :],
                                    vaug[:, m * 260 + 65 * hA : m * 260 + 65 * hA + 65],
                                    ptA[:, m * 512 : (m + 1) * 512],
                                    start=(m == 0),
                                    stop=(m == MT - 1),
                                )
                                mm(
                                    oB[0:65, :],
                                    vaug[:, m * 260 + 65 * hB : m * 260 + 65 * hB + 65],
                                    ptB[:, m * 512 : (m + 1) * 512],
                                    start=(m == 0),
                                    stop=(m == MT - 1),
                                )
                        # normalize: row 64 of o[AB] is Z = sum_m exp(S^T)
                        for hx, ops in ((hA, oA), (hB, oB)):
                            rz = small.tile([128, 512], F32, tag="rz", name=f"rz{hx}")
                            nc.vector.reciprocal(
                                out=rz[64:65, :], in_=ops[64:65, :]
                            )
                            bc = ps_y.tile([128, 512], F32, tag="yps", name=f"bc{hx}")
                            nc.tensor.matmul(
                                bc[0:64, :],
                                ones_sb[64:65, 0:64],
                                rz[64:65, :],
                                start=True,
                                stop=True,
                            )
                            bcs = small.tile([128, 512], F32, tag="bcs", name=f"bcs{hx}")
                            nc.vector.tensor_copy(bcs[0:64, :], bc[0:64, :])
                            nc.vector.tensor_mul(
                                onorm[hx][:, ch * 512 : (ch + 1) * 512],
                                ops[0:64, :],
                                bcs[0:64, :],
                            )

            # ---------------- phase D: output projection ----------------
            with (
                tc.tile_pool(name="ps_y", bufs=6, space="PSUM") as ps_y,
                tc.tile_pool(name="yout", bufs=4) as yout,
            ):
                for st in range(MT):
                    for fc in range(2):
                        yps = ps_y.tile([128, 512], F32, tag="yps")
                        for h in range(HPC):
                            mm(
                                yps,
                                onorm[h][:, st * 128 : (st + 1) * 128],
                                wp_sb[:, h * C + fc * 512 : h * C + (fc + 1) * 512],
                                start=(h == 0),
                                stop=(h == HPC - 1),
                            )
                        ysb = yout.tile([128, 512], F32, tag="ysb")
                        nc.vector.tensor_copy(ysb, yps)
                        nc.sync.dma_start(
                            out=y_d[st * 128 : (st + 1) * 128, fc * 512 : (fc + 1) * 512],
                            in_=ysb,
                        )

    nc.compile()
    return nc


def make_core_inputs(x, Wqkv, Wproj):
    """Per-core input dicts. Core c: batch c//4, heads 4*(c%4) .. 4*(c%4)+3."""
    scale = D**-0.5
    xts = [np.ascontiguousarray(x[b].T).astype(np.float32) for b in range(B)]
    in_maps = []
    for core in range(8):
        b, hg = core // 4, core % 4
        heads = [HPC * hg + i for i in range(HPC)]
        rows_q = np.concatenate([Wqkv[D * h : D * (h + 1)] for h in heads]) * scale
        rows_k = np.concatenate([Wqkv[C + D * h : C + D * (h + 1)] for h in heads])
        wqk = np.ascontiguousarray(np.concatenate([rows_q, rows_k]).T, dtype=np.float32)
        wv = np.ascontiguousarray(
            np.concatenate([Wqkv[2 * C + D * h : 2 * C + D * (h + 1)] for h in heads]).T,
            dtype=np.float32,
        )
        wp = np.ascontiguousarray(
            np.concatenate([Wproj[:, D * h : D * (h + 1)] for h in heads], axis=1).T,
            dtype=np.float32,
        )
        in_maps.append({"xt": xts[b], "wqk": wqk, "wv": wv, "wp": wp})
    return in_maps


_EXEC_CACHE = {}


def _get_executor():
    """Build + jit the 8-core SPMD executable once per process."""
    if "fn" in _EXEC_CACHE:
        return _EXEC_CACHE
    import jax
    from jax.sharding import Mesh, PartitionSpec
    from jax.experimental.shard_map import shard_map
    from concourse import bass2jax
    from concourse.bass2jax import _bass_exec_p, partition_id_tensor

    nc = build_bass()
    bass2jax.install_neuronx_cc_hook()
    pid = nc.partition_id_tensor.name if nc.partition_id_tensor else None
    in_names, out_names, out_avals = [], [], []
    for alloc in nc.m.functions[0].allocations:
        if not isinstance(alloc, mybir.MemoryLocationSet):
            continue
        name = alloc.memorylocations[0].name
        if alloc.kind == "ExternalInput":
            if name != pid:
                in_names.append(name)
        elif alloc.kind == "ExternalOutput":
            out_names.append(name)
            out_avals.append(
                jax.core.ShapedArray(
                    tuple(alloc.tensor_shape), mybir.dt.np(alloc.dtype)
                )
            )
    n_params = len(in_names)
    all_names = list(in_names) + list(out_names) + ([pid] if pid else [])

    def body(*args):
        *ins, yb = args
        operands = list(ins) + [yb]
        if pid:
            operands.append(partition_id_tensor())
        outs = _bass_exec_p.bind(
            *operands,
            out_avals=tuple(out_avals),
            in_names=tuple(all_names),
            out_names=tuple(out_names),
            lowering_input_output_aliases=(),
            sim_require_finite=True,
            sim_require_nnan=True,
            nc=nc,
        )
        return outs[0]

    mesh = Mesh(np.asarray(jax.devices()[:8]), ("core",))
    fn = jax.jit(
        shard_map(
            body,
            mesh=mesh,
            in_specs=(PartitionSpec("core"),) * (n_params + 1),
            out_specs=PartitionSpec("core"),
            check_rep=False,
        ),
        donate_argnums=(n_params,),
    )
    _EXEC_CACHE.update(fn=fn, in_names=in_names)
    return _EXEC_CACHE


def kernel(x, Wqkv, Wproj, bproj):
    x = np.asarray(x, dtype=np.float32)
    Wqkv = np.asarray(Wqkv, dtype=np.float32)
    Wproj = np.asarray(Wproj, dtype=np.float32)
    bproj = np.asarray(bproj, dtype=np.float32)

    ex = _get_executor()
    in_maps = make_core_inputs(x, Wqkv, Wproj)
    glob_ins = [
        np.concatenate([np.asarray(m[name]) for m in in_maps], axis=0)
        for name in ex["in_names"]
    ]
    y0 = np.zeros((8 * S, C), np.float32)
    out = np.asarray(ex["fn"](*glob_ins, y0))  # [8*S, C]

    y = np.zeros((B, S, C), dtype=np.float32)
    for core in range(8):
        y[core // 4] += out[core * S : (core + 1) * S, :]
    y += bproj
    return y
